# revision 12
# baseline (speedup 1.0000x reference)
"""Local2DAttention TRN2 kernel (nn_Local2DAttention_79207786873330).

Math (faithful to the reference's torch-bug semantics):
  x (16, 1024, 512) is window-blocked into M=256 "windows" (b, i, j) of 8x8
  spatial positions. A plain row-major reshape of each (E, 8, 8) block into
  (64, 512) scrambles channels/spatial into 64 tokens per window:
      y[m, t, e] = x[b, (i*8+w1)*32 + j*8 + w2, 8t+a],  e = a*64 + w1*8 + w2
  nn.MultiheadAttention (batch_first=False) then attends over the M=256 axis
  with the 64 t-positions as batch and 8 heads:
      per (t, h): S = Q K^T / 8 over 256x256, softmax, O = P V.

Sharding: the 64 t-positions split 8 per core (t = 8*cc + tl). Attention,
projections and output assembly are fully independent per t -> zero
cross-core communication. Weights are replicated.

Device pipeline per core (bf16 matmul operands, fp32 accumulation):
  yT (512, 2048)      - host-permuted token matrix, f-major (e x tokens)
  QK^T proj: PROJ^T[r, tok] = Wqk^T.T @ yT  (q rows pre-scaled by 1/8)
  V    proj: V'[tok, 8x66]  = yT.T @ Wv''   (66-stride heads, ones col at 64)
  per (t, h):  S^T = K^T.T @ Q^T  -> exp (ACT, ->bf16)
               pso = V'[:, h-slice].T @ P^T   (65 rows: 64 O_u rows + D row)
               rc = approx-recip(D row)  (DVE, PSUM read, 1 op per (t,hp))
               rb = gpsimd partition_broadcast(rc) -> SBUF (64, 512)
               ot = pso[0:64] * rb  (DVE, PSUM operand, direct normalize)
  out proj: Z = O^T.T @ Wout^T + b_eff  (b_eff = b_out + Wout @ b_v, host)
"""
import sys
sys.path.insert(0, '/opt/trn_rl_repo')
import numpy as np
import ml_dtypes

BF = ml_dtypes.bfloat16




# problem constants (hardcoded per contract)
B, N, E = 16, 1024, 512
WIN = 8          # window_size
HS = 4           # hS = S // W,  S = 32
NH = 8           # heads
HD = 64          # head dim
NCORES = 8
TL = 8           # t-values per core
MTOK = 256       # windows (= B*HS*HS) = tokens per t
TOK = TL * MTOK  # tokens per core

_cache = {}


def _split_multiwaits(nc, mybir, limit=1):
    """This toolchain's walrus encodes at most one semaphore wait per
    instruction; hoist excess waits into preceding NoOps on the same engine."""
    n_split = 0
    for f in nc.m.functions:
        for blk in f.blocks:
            insts = blk.instructions
            out = []
            for inst in insts:
                si = inst.sync_info
                waits = list(si.on_wait) if (si is not None and si.on_wait) else []
                if len(waits) > limit:
                    excess, keep = waits[:-limit], waits[-limit:]
                    for w in excess:
                        nop = mybir.InstNoOp(
                            name=f"{inst.name}-wsplit{n_split}",
                            engine=inst.engine,
                            ins=[], outs=[],
                            sync_info=mybir.SyncInfo(on_wait=[w], on_update=[]),
                        )
                        out.append(nop)
                        n_split += 1
                    inst.sync_info = mybir.SyncInfo(
                        on_wait=keep, on_update=list(si.on_update or []))
                out.append(inst)
            if n_split:
                insts.clear()
                insts.extend(out)
    return n_split


def _build_module(split_waits=True):
    import concourse.bass as bass
    import concourse.mybir as mybir
    from concourse import tile, library_config
    from concourse.library_overlay import lower_extended_insts

    f32 = mybir.dt.float32
    bf16 = mybir.dt.bfloat16
    Exp = mybir.ActivationFunctionType.Exp
    Ident = mybir.ActivationFunctionType.Identity

    nc = bass.Bass()
    YT = nc.dram_tensor("yT", [E, TOK], bf16, kind="ExternalInput")
    WQK = nc.dram_tensor("wqk", [E, 2 * E], bf16, kind="ExternalInput")
    WV = nc.dram_tensor("wv", [E, NH * 66], bf16, kind="ExternalInput")
    WO = nc.dram_tensor("wo", [E, E], bf16, kind="ExternalInput")
    BQK = nc.dram_tensor("bqk", [128, 8], f32, kind="ExternalInput")
    BEFF = nc.dram_tensor("beff", [1, E], f32, kind="ExternalInput")
    OUT = nc.dram_tensor("o", [TOK, E], f32, kind="ExternalOutput")

    with tile.TileContext(nc) as tc:
        with (
            tc.tile_pool(name="persist", bufs=1) as pers,
            tc.tile_pool(name="qk", bufs=2) as qkp,
            tc.tile_pool(name="v", bufs=2) as vpool,
            tc.tile_pool(name="pt", bufs=6) as ptp,
            tc.tile_pool(name="ot", bufs=6) as otp,
            tc.tile_pool(name="sm", bufs=6) as smp,
            tc.tile_pool(name="rb", bufs=6) as rbp,
            tc.tile_pool(name="z", bufs=4) as zp,
            tc.tile_pool(name="ps", bufs=8, space="PSUM") as psp,
        ):
            # gpsimd ucode library with partition_broadcast (attn, index 1);
            # must execute before the first broadcast on the Pool queue.
            nc.gpsimd.load_library(library_config.attn)
            # HAM warmup: the PE clock sits at 1.2 GHz until ~3.4us of
            # sustained matmul activity. Burn dummy matmuls on scratch data
            # during the input-DMA wait so the real projections start warm.
            wsc = pers.tile([128, 512], bf16, tag="wsc")
            nc.vector.memset(wsc[:], 0.5)
            for wi in range(14):
                psw = psp.tile([128, 512], f32, tag="ps", name=f"psw_{wi}")
                nc.tensor.matmul(psw[:], wsc[:, 0:128], wsc[:],
                                 start=True, stop=True)
            # persistent loads. Order DMA issues by first use: token chunk 0
            # of yT + the QK weights unblock the first projection; the rest
            # streams in behind it. All input loads issue on the sync queue;
            # scalar queue gets a few so ACT isn't blocked later.
            yts, wqks, wvs, wos = [], [], [], []
            for fi in range(4):
                yt = pers.tile([128, TOK], bf16, tag=f"yt{fi}")
                nc.sync.dma_start(yt[:, 0:512], YT[fi * 128:(fi + 1) * 128, 0:512])
                yts.append(yt)
            for fi in range(4):
                w = pers.tile([128, 2 * E], bf16, tag=f"wqk{fi}")
                nc.scalar.dma_start(w[:], WQK[fi * 128:(fi + 1) * 128, :])
                wqks.append(w)
            bqks = pers.tile([128, 8], f32, tag="bqk")
            nc.scalar.dma_start(bqks[:], BQK[:])
            for fi in range(4):
                w = pers.tile([128, NH * 66], bf16, tag=f"wv{fi}")
                nc.scalar.dma_start(w[:], WV[fi * 128:(fi + 1) * 128, :])
                wvs.append(w)
            for ck in range(1, 4):
                for fi in range(4):
                    nc.sync.dma_start(
                        yts[fi][:, ck * 512:(ck + 1) * 512],
                        YT[fi * 128:(fi + 1) * 128, ck * 512:(ck + 1) * 512])
            for fi in range(4):
                w = pers.tile([128, E], bf16, tag=f"wo{fi}")
                nc.sync.dma_start(w[:], WO[fi * 128:(fi + 1) * 128, :])
                wos.append(w)
            beffb = pers.tile([128, E], f32, tag="beffb")
            nc.sync.dma_start(beffb[:], BEFF[:].partition_broadcast(128).squeeze(1))

            def proj(tp):
                """QK^T + V' projections for t-pair tp. Returns (qk, vts)."""
                ptok0 = tp * 2 * MTOK
                qk = []
                for ri in range(8):
                    ps = psp.tile([128, 2 * MTOK], f32, tag="ps",
                                  name=f"psq{ri}_{tp}")
                    for fi in range(4):
                        nc.tensor.matmul(
                            ps[:],
                            wqks[fi][:, ri * 128:(ri + 1) * 128],
                            yts[fi][:, ptok0:ptok0 + 2 * MTOK],
                            start=(fi == 0), stop=(fi == 3))
                    qt = qkp.tile([128, 2 * MTOK], bf16, tag=f"qk{ri}",
                                  name=f"qk{ri}_{tp}")
                    # PSUM->SBUF eviction + per-partition bias; alternate
                    # engines so neither DVE nor ACT becomes the bottleneck.
                    if ri % 2 == 0:
                        nc.vector.tensor_scalar_add(qt[:], ps[:], bqks[:, ri:ri + 1])
                    else:
                        nc.scalar.activation(qt[:], ps[:], Ident,
                                             bias=bqks[:, ri:ri + 1])
                    qk.append(qt)
                vts = []
                for sc in range(4):
                    vt = vpool.tile([128, NH * 66], bf16, tag=f"v{sc}",
                                    name=f"v{sc}_{tp}")
                    psvs = [psp.tile([128, NH * 66 // 2], f32, tag="ps",
                                     name=f"psv{half}_{sc}_{tp}")
                            for half in range(2)]
                    for fi in range(4):
                        for half in range(2):
                            c0 = half * (NH * 66 // 2)  # 264
                            nc.tensor.matmul(
                                psvs[half][:],
                                yts[fi][:, ptok0 + sc * 128:ptok0 + (sc + 1) * 128],
                                wvs[fi][:, c0:c0 + NH * 66 // 2],
                                start=(fi == 0), stop=(fi == 3))
                    for half in range(2):
                        c0 = half * (NH * 66 // 2)
                        nc.scalar.copy(vt[:, c0:c0 + NH * 66 // 2], psvs[half][:])
                    ones = vt[:].rearrange("p (h c) -> p h c", h=NH)[:, :, 64:65]
                    nc.gpsimd.memset(ones, 1.0)
                    vts.append(vt)
                return qk, vts

            def attention(tp, qk, vts):
                """Attention + out-projection for t-pair tp."""
                ot_all = {}
                for hp in range(4):          # head-pairs, both t's interleaved
                    for ti in range(2):
                        t = tp * 2 + ti
                        # scores: sc outer / hh inner so consecutive matmuls
                        # target alternating PE row halves (tile_position 0/64)
                        # -> they execute concurrently in the array.
                        psss = []
                        for hh in range(2):
                            psss.append(psp.tile([128, 2 * MTOK], f32, tag="ps",
                                                 name=f"pss_{t}_{hp}_{hh}"))
                        for sc in range(2):
                            for hh in range(2):
                                ho = hh * 64
                                nc.tensor.matmul(
                                    psss[hh][:, sc * MTOK:(sc + 1) * MTOK],
                                    qk[4 + hp][ho:ho + 64,
                                               ti * MTOK + sc * 128:ti * MTOK + (sc + 1) * 128],
                                    qk[hp][ho:ho + 64, ti * MTOK:(ti + 1) * MTOK],
                                    start=True, stop=True, tile_position=(ho, 0),
                                    skip_group_check=True)
                        pts = []
                        for hh in range(2):
                            pt = ptp.tile([128, 2 * MTOK], bf16, tag=f"pt{hh}",
                                          name=f"pt{hh}_{t}_{hp}")
                            nc.scalar.activation(pt[:], psss[hh][:], Exp)
                            pts.append(pt)
                        # AV for both heads into one (65,512) bank (serial,
                        # disjoint columns). Row 64 is the softmax denominator
                        # (ones column of V'). Normalize without evicting:
                        # approx-recip of the D row, gpsimd broadcast to 64
                        # partitions in SBUF, DVE multiply straight from PSUM.
                        pso = psp.tile([65, 2 * MTOK], f32, tag="ps",
                                       name=f"pso_{t}_{hp}")
                        for hh in range(2):
                            h = 2 * hp + hh
                            for sc in range(2):
                                nc.tensor.matmul(
                                    pso[:, hh * MTOK:(hh + 1) * MTOK],
                                    vts[2 * ti + sc][:, h * 66:h * 66 + 65],
                                    pts[hh][:, sc * MTOK:(sc + 1) * MTOK],
                                    start=(sc == 0), stop=(sc == 1),
                                    skip_group_check=True)
                        # the custom-DVE approx recip only works at base
                        # partition 0 -> stage the D row there first (1-lane
                        # copy; still 3x cheaper than exact recip)
                        dr = smp.tile([1, 2 * MTOK], f32, tag="dr",
                                      name=f"dr_{t}_{hp}")
                        nc.vector.tensor_copy(dr[:], pso[64:65, :])
                        rc = smp.tile([1, 2 * MTOK], f32, tag="rc",
                                      name=f"rc_{t}_{hp}")
                        with nc.allow_low_precision(
                                reason="approx 1/D (~18 bits); D in [150,400]"):
                            nc.vector.reciprocal_approx_fast(
                                out=rc[:], in_=dr[:])
                        rb = rbp.tile([64, 2 * MTOK], f32, tag="rb",
                                      name=f"rb_{t}_{hp}")
                        nc.gpsimd.partition_broadcast(rb[:], rc[:], channels=64)
                        ot = otp.tile([128, MTOK], bf16, tag=f"ot{hp}",
                                      name=f"ot{hp}_{t}")
                        for hh in range(2):
                            nc.vector.tensor_mul(
                                ot[hh * 64:(hh + 1) * 64, :],
                                pso[0:64, hh * MTOK:(hh + 1) * MTOK],
                                rb[:, hh * MTOK:(hh + 1) * MTOK])
                        ot_all[(ti, hp)] = ot
                for ti in range(2):
                    t = tp * 2 + ti
                    tok0 = t * MTOK
                    for lc in range(2):
                        psz = psp.tile([128, E], f32, tag="ps",
                                       name=f"psz_{t}_{lc}")
                        for fi in range(4):
                            nc.tensor.matmul(
                                psz[:], ot_all[(ti, fi)][:, lc * 128:(lc + 1) * 128],
                                wos[fi][:],
                                start=(fi == 0), stop=(fi == 3))
                        zt = zp.tile([128, E], f32, tag="zt", name=f"zt_{t}_{lc}")
                        nc.vector.tensor_add(zt[:], psz[:], beffb[:])
                        nc.sync.dma_start(
                            OUT[tok0 + lc * 128:tok0 + (lc + 1) * 128, :], zt[:])

            # software pipeline: emit tp+1's dense projection matmuls BEFORE
            # tp's attention groups, so the PE engine queue always has dense
            # work while tp's exp/recip/broadcast chains drain on ACT/DVE/Pool.
            pq = proj(0)
            for tp in range(TL // 2):
                nxt = proj(tp + 1) if tp + 1 < TL // 2 else None
                attention(tp, *pq)
                pq = nxt

    if split_waits:
        _split_multiwaits(nc, mybir)
    lower_extended_insts(nc)  # encode InstPartitionBroadcast bytes
    return nc


def _host_prep(x, in_proj_w, in_proj_b, out_proj_w, out_proj_b):
    x = np.asarray(x, dtype=np.float32)
    in_proj_w = np.asarray(in_proj_w, dtype=np.float32)
    in_proj_b = np.asarray(in_proj_b, dtype=np.float32)
    out_proj_w = np.asarray(out_proj_w, dtype=np.float32)
    out_proj_b = np.asarray(out_proj_b, dtype=np.float32)

    # weights (replicated); fold the 1/sqrt(hd)=1/8 score scale into q rows
    wq = in_proj_w[:E] / 8.0
    wk = in_proj_w[E:2 * E]
    wv = in_proj_w[2 * E:]
    wqk = np.concatenate([wq, wk], 0).T.copy().astype(BF)        # (512, 1024)
    wv66 = np.zeros((E, NH * 66), dtype=np.float32)              # (512, 528)
    for h in range(NH):
        wv66[:, h * 66:h * 66 + 64] = wv[h * 64:(h + 1) * 64].T
    wv66 = wv66.astype(BF)
    wo = out_proj_w.T.copy().astype(BF)                          # (512, 512)
    bqk = np.concatenate([in_proj_b[:E] / 8.0, in_proj_b[E:2 * E]])
    bqk = bqk.reshape(8, 128).T.copy().astype(np.float32)        # (128, 8)
    beff = (out_proj_b + out_proj_w @ in_proj_b[2 * E:]).reshape(1, E)
    beff = beff.astype(np.float32)

    # per-core token matrices: yT[f=(a,w1,w2), col=(tl, b, i, j)]
    # channel c = 64*cc + 8*tl + a  (t = 8*cc + tl)
    xv = x.reshape(B, HS, WIN, HS, WIN, NCORES, TL, WIN)  # b i w1 j w2 cc tl a
    yts = []
    for cc in range(NCORES):
        yt = xv[:, :, :, :, :, cc].transpose(6, 2, 4, 5, 0, 1, 3)
        yts.append(np.ascontiguousarray(yt).reshape(E, TOK).astype(BF))
    return yts, wqk, wv66, wo, bqk, beff


def kernel(x, in_proj_w, in_proj_b, out_proj_w, out_proj_b,
           window_size=8, nhead=8, **_unused):
    from concourse.bass_utils import run_bass_kernel_spmd

    yts, wqk, wv66, wo, bqk, beff = _host_prep(
        x, in_proj_w, in_proj_b, out_proj_w, out_proj_b)

    if "nc" not in _cache:
        _cache["nc"] = _build_module()
    nc = _cache["nc"]

    in_maps = [
        {"yT": yts[cc], "wqk": wqk, "wv": wv66, "wo": wo,
         "bqk": bqk, "beff": beff}
        for cc in range(NCORES)
    ]
    res = run_bass_kernel_spmd(nc, in_maps, core_ids=list(range(NCORES)))

    out = np.empty((B, N, E), dtype=np.float32)
    ov = out.reshape(B, HS, WIN, HS, WIN, E)  # b i w1 j w2 e
    for cc in range(NCORES):
        z = res.results[cc]["o"].reshape(TL, B, HS, HS, E)  # tl b i j e
        # t = 8*cc + tl -> w1 = cc, w2 = tl
        ov[:, :, cc, :, :, :] = z.transpose(1, 2, 3, 0, 4)
    return out
